# revision 7
# baseline (speedup 1.0000x reference)
"""Trainium2 Bass kernel for a Mixtral decoder layer (attention + top-2 MoE).

Contract: kernel(**inputs) takes the FULL unsharded inputs (as produced by
reference.setup_inputs()) and returns the full outputs (out, residual), both
[B, S, D] float32.

Sharding across the 8 NeuronCores:
  Phase 1 (attention): tensor-parallel over heads. Each core owns 2 q-heads +
  1 kv-head (colwise qkv slice) and the matching 256-column slice of wo
  (rowwise o_proj). Cores emit o_proj partial sums [T, D]; the host combines
  them (the all-reduce step) and applies the residual add + post-attention
  RMSNorm + router on the host (tiny fraction of total FLOPs).
  Phase 2 (MoE): expert-parallel. Core e owns expert e's weights; the host
  gathers the tokens routed to each expert (capacity-padded), each core runs
  the SwiGLU expert densely, and the host scatter-adds the weighted results.

Precision split: phase 1 runs f32/f32r end-to-end because the host-side
router (top-2 over gate logits) is computed from the phase-1 residual --
even ~1e-4 relative error there flips expert assignments vs the reference
and costs ~2e-2 output error. Phase 2 runs bf16 (same PE rate, half the
DMA): its error (~4e-3) only perturbs the final output linearly.

Attention avoids all probability transposes by computing scores transposed
(k^T q), exponentiating without max-subtraction (scores are bounded ~|5|:
wqkv scale 0.02 over D=2048 gives score std ~0.8), masking multiplicatively
with a 0/1 mask after exp, reducing the softmax denominator with a ones-
column matmul, and broadcasting 1/l across partitions with a rank-1 matmul.
"""

import math
from functools import lru_cache

import numpy as np
import ml_dtypes

import concourse.bass as bass
import concourse.mybir as mybir
import concourse.tile as tile
from concourse import bacc
from concourse import bass_utils

# ---- problem shapes (hardcoded per contract) ----
B, S, D = 2, 2048, 2048
NH, NKV, HD = 16, 8, 128
E, TOPK, F = 8, 2, 4096
EPS = 1e-5
T = B * S
NCORES = 8
P = 128

F32 = mybir.dt.float32
F32R = mybir.dt.float32r
BF16 = mybir.dt.bfloat16
NPBF = ml_dtypes.bfloat16
DKT = D // P   # 16 k-tiles over D
FBT = F // P   # 32 f-blocks over F


def _chunks(n, hi=512):
    out = []
    off = 0
    while off < n:
        c = min(hi, n - off)
        out.append((off, c))
        off += c
    return out


# ---------------------------------------------------------------- phase 2
@lru_cache(maxsize=None)
def build_phase2(C, reps=1):
    """Per-core SwiGLU expert over C capacity-padded tokens (bf16).

    Inputs (per core, host-pretransposed so every DMA is contiguous):
      xh  [P, DKT, C]      xh[p,k,t]      = x[t, k*128+p]
      w13 [P, FBT, 2, DKT, P] w13[p,fb,s,k,f] = w{1,3}[fb*128+f, k*128+p]
      w2h [P, DKT, FBT, P] w2h[p,dm,fi,d] = w2[dm*128+d, fi*128+p]
    Output:
      y   [P, DKT, C] f32  y[p,dm,t]      = y_full[dm*128+p, t]
    """
    nch = _chunks(C)

    nc = bacc.Bacc(None, target_bir_lowering=False, debug=False)
    with tile.TileContext(nc) as tc:
        with (
            tc.tile_pool(name="dram", bufs=1, space="DRAM") as dram,
            tc.tile_pool(name="xp", bufs=1) as xp,
            tc.tile_pool(name="gup", bufs=1) as gup,
            tc.tile_pool(name="wp", bufs=3) as wp,
            tc.tile_pool(name="w2p", bufs=2) as w2p,
            tc.tile_pool(name="gt", bufs=4) as gtp,
            tc.tile_pool(name="ps_g", bufs=4, space="PSUM") as ps_g,
            tc.tile_pool(name="ps_y", bufs=4, space="PSUM") as ps_y,
        ):
            xh = dram.tile([P, DKT, C], BF16, kind="ExternalInput", name="xh", uniquify=False)
            w13 = dram.tile([P, FBT, 2, DKT, P], BF16, kind="ExternalInput", name="w13", uniquify=False)
            w2h = dram.tile([P, DKT, FBT, P], BF16, kind="ExternalInput", name="w2h", uniquify=False)
            y = dram.tile([P, DKT, C], F32, kind="ExternalOutput", name="y", uniquify=False)

            def body():
                x_sb = xp.tile([P, DKT, C], BF16, tag="x")
                for k in range(DKT):
                    nc.sync.dma_start(x_sb[:, k], xh[:, k])
                gu = gup.tile([P, FBT, C], BF16, tag="gu")

                for fb in range(FBT):
                    wc = wp.tile([P, 2, DKT, P], BF16, tag="w13")
                    nc.sync.dma_start(wc[:], w13[:, fb])
                    for (n0, nw) in nch:
                        pg = ps_g.tile([P, 512], F32, tag="pg", name="pg")
                        pu = ps_g.tile([P, 512], F32, tag="pg", name="pu")
                        for k in range(DKT):
                            nc.tensor.matmul(
                                pg[:, :nw], wc[:, 0, k], x_sb[:, k, n0:n0 + nw],
                                start=(k == 0), stop=(k == DKT - 1))
                        for k in range(DKT):
                            nc.tensor.matmul(
                                pu[:, :nw], wc[:, 1, k], x_sb[:, k, n0:n0 + nw],
                                start=(k == 0), stop=(k == DKT - 1))
                        g = gtp.tile([P, 512], BF16, tag="g")
                        nc.scalar.activation(
                            g[:, :nw], pg[:, :nw],
                            mybir.ActivationFunctionType.Silu)
                        nc.vector.tensor_mul(
                            gu[:, fb, n0:n0 + nw], g[:, :nw], pu[:, :nw])

                for dm in range(DKT):
                    w2c = w2p.tile([P, FBT, P], BF16, tag="w2")
                    nc.sync.dma_start(w2c[:], w2h[:, dm])
                    for (n0, nw) in nch:
                        py = ps_y.tile([P, 512], F32, tag="py")
                        for fi in range(FBT):
                            nc.tensor.matmul(
                                py[:, :nw], w2c[:, fi], gu[:, fi, n0:n0 + nw],
                                start=(fi == 0), stop=(fi == FBT - 1))
                        yt = gtp.tile([P, 512], F32, tag="yt")
                        nc.vector.tensor_copy(yt[:, :nw], py[:, :nw])
                        nc.sync.dma_start(y[:, dm, n0:n0 + nw], yt[:, :nw])

            if reps == 1:
                body()
            else:
                with tc.For_i(0, reps, 1):
                    body()
    nc.compile()
    return nc


def _pad_to(x, n, axis=0):
    pad = [(0, 0)] * x.ndim
    pad[axis] = (0, n - x.shape[axis])
    return np.pad(x, pad)


@lru_cache(maxsize=None)
def _phase2_weights(_key=0):
    return None  # placeholder (weights prepped in phase2_in_maps)


def phase2_capacity(tok_idx):
    max_ne = max(len(ix) for ix in tok_idx)
    return max(128, ((max_ne + 127) // 128) * 128)


def phase2_in_maps(h2, tok_idx, w1, w3, w2, C):
    """h2: [T, D] f32 routed input. tok_idx: list of E index arrays."""
    in_maps = []
    for e in range(E):
        xe = _pad_to(h2[tok_idx[e]], C, axis=0)          # [C, D]
        xh = np.ascontiguousarray(
            xe.astype(NPBF).reshape(C, DKT, P).transpose(2, 1, 0))
        w1b = w1[e].astype(NPBF).reshape(FBT, P, DKT, P).transpose(3, 0, 2, 1)
        w3b = w3[e].astype(NPBF).reshape(FBT, P, DKT, P).transpose(3, 0, 2, 1)
        w13 = np.ascontiguousarray(np.stack([w1b, w3b], axis=2))
        w2b = np.ascontiguousarray(
            w2[e].astype(NPBF).reshape(DKT, P, FBT, P).transpose(3, 0, 2, 1))
        in_maps.append({"xh": xh, "w13": w13, "w2h": w2b})
    return in_maps


def run_phase2(h2, tok_idx, w1, w3, w2, reps=1):
    """Returns list of y_e [n_e, D] f32 (unweighted expert outputs)."""
    C = phase2_capacity(tok_idx)
    nc = build_phase2(C, reps)
    in_maps = phase2_in_maps(h2, tok_idx, w1, w3, w2, C)
    res = bass_utils.run_bass_kernel_spmd(nc, in_maps, core_ids=list(range(NCORES)))
    outs = []
    for e in range(E):
        ye = res.results[e]["y"]                         # [P, DKT, C]
        yfull = ye.transpose(1, 0, 2).reshape(D, C)
        outs.append(np.ascontiguousarray(yfull.T[: len(tok_idx[e])]))
    return outs


# ---------------------------------------------------------------- phase 1
ST = S // P            # 16 seq tiles per batch
SC = S // 512          # 4 seq chunks of 512 per batch
QH = 2                 # q-heads per core


@lru_cache(maxsize=None)
def build_phase1(reps=1):
    """Per-core attention slice: 2 q-heads + 1 kv-head, both batches.

    Scores are computed transposed (k on partitions), exp'd without
    max-subtraction, masked multiplicatively post-exp, and consumed
    directly by the AV matmul -- no probability transposes. The softmax
    denominator comes from a ones-column matmul accumulated alongside AV;
    1/l is broadcast across partitions with a rank-1 matmul.
    """
    nc = bacc.Bacc(None, target_bir_lowering=False, debug=False)
    from concourse.masks import make_identity

    with tile.TileContext(nc) as tc:
        with (
            tc.tile_pool(name="dram", bufs=1, space="DRAM") as dram,
            tc.tile_pool(name="const", bufs=1) as constp,
            tc.tile_pool(name="xs", bufs=3) as xs,
            tc.tile_pool(name="rt", bufs=1) as rtp,
            tc.tile_pool(name="ptp", bufs=3) as ptp,
            tc.tile_pool(name="stat", bufs=3) as statp,
            tc.tile_pool(name="oout", bufs=4) as oout,
            tc.tile_pool(name="ps_a", bufs=4, space="PSUM") as ps_a,
            tc.tile_pool(name="ps_b", bufs=2, space="PSUM") as ps_b,
            tc.tile_pool(name="ps_c", bufs=2, space="PSUM") as ps_c,
        ):
            XH = dram.tile([P, DKT, T], F32R, kind="ExternalInput", name="xh", uniquify=False)
            WQ = dram.tile([P, DKT, 4 * P], F32R, kind="ExternalInput", name="wq", uniquify=False)
            WO = dram.tile([P, QH, D], F32R, kind="ExternalInput", name="wo", uniquify=False)
            CS = dram.tile([P, T], F32R, kind="ExternalInput", name="cs", uniquify=False)
            SSG = dram.tile([P, T], F32R, kind="ExternalInput", name="ss", uniquify=False)
            STOK = dram.tile([P, T // P], F32, kind="ExternalInput", name="stok", uniquify=False)
            MASK = dram.tile([P, 5, 512], F32R, kind="ExternalInput", name="masks", uniquify=False)
            PO = dram.tile([T, D], F32, kind="ExternalOutput", name="po", uniquify=False)

            def body():
                wq_sb = constp.tile([P, DKT, 4 * P], F32R, tag="wq")
                nc.sync.dma_start(wq_sb[:], WQ[:])
                wo_sb = constp.tile([P, QH, D], F32R, tag="wo")
                nc.sync.dma_start(wo_sb[:], WO[:])
                cs_sb = constp.tile([P, T], F32R, tag="cs")
                nc.sync.dma_start(cs_sb[:], CS[:])
                ss_sb = constp.tile([P, T], F32R, tag="ss")
                nc.sync.dma_start(ss_sb[:], SSG[:])
                stok_sb = constp.tile([P, T // P], F32, tag="stok")
                nc.sync.dma_start(stok_sb[:], STOK[:])
                mask_sb = constp.tile([P, 5, 512], F32R, tag="mask")
                nc.sync.dma_start(mask_sb[:], MASK[:])
                ident = constp.tile([P, P], F32, tag="ident")
                make_identity(nc, ident[:])
                ones_col = mask_sb[:, 4, 0:1]
                ones_row = mask_sb[0:1, 4, 0:P]

                for b in range(B):
                    toff = b * S
                    q_r = [rtp.tile([P, S], F32R, tag=f"qr{h}", name=f"q_r{h}")
                           for h in range(QH)]
                    k_r = rtp.tile([P, S], F32R, tag="kr")
                    v_tm = rtp.tile([P, ST, P], F32R, tag="vtm")
                    attn_f = [rtp.tile([P, S], F32R, tag=f"af{h}", name=f"attn{h}")
                              for h in range(QH)]

                    # ---- qkv projection + rope + v transpose ----
                    for n in range(SC):
                        nsl = slice(toff + n * 512, toff + (n + 1) * 512)
                        pq = [ps_a.tile([P, 512], F32, tag="mm", name=f"pq{m}")
                              for m in range(4)]
                        for k in range(DKT):
                            xt = xs.tile([P, 512], F32R, tag="xt")
                            nc.sync.dma_start(xt[:], XH[:, k, nsl])
                            for m in range(4):
                                nc.tensor.matmul(
                                    pq[m][:], wq_sb[:, k, m * P:(m + 1) * P], xt[:],
                                    start=(k == 0), stop=(k == DKT - 1))
                        # rope for q0, q1, k (ss rows 0:64 pre-negated on host)
                        for m in range(3):
                            dst = (q_r[m] if m < QH else k_r)[:, n * 512:(n + 1) * 512]
                            tmp = statp.tile([P, 512], F32, tag="rtmp")
                            nc.vector.tensor_mul(tmp[:64], pq[m][64:], ss_sb[:64, nsl])
                            nc.vector.tensor_mul(tmp[64:], pq[m][:64], ss_sb[64:, nsl])
                            nc.vector.tensor_mul(dst, pq[m][:], cs_sb[:, nsl])
                            nc.vector.tensor_add(dst, dst, tmp[:])
                        # v: evict, transpose to token-major, scale by stok
                        vst = statp.tile([P, 512], F32, tag="vst")
                        nc.vector.tensor_copy(vst[:], pq[3][:])
                        for j in range(4):
                            tt = n * 4 + j
                            trp = ps_b.tile([P, P], F32, tag="bav", name="tr")
                            nc.tensor.transpose(
                                trp[:], vst[:, j * P:(j + 1) * P], ident[:])
                            nc.vector.tensor_scalar_mul(
                                v_tm[:, tt, :], trp[:],
                                stok_sb[:, b * ST + tt:b * ST + tt + 1])

                    # ---- attention (transposed scores, pipelined) ----
                    for h in range(QH):
                        sched = [(qb, kt) for qb in range(SC)
                                 for kt in range(4 * (qb + 1))]
                        state = {}

                        def emit_scores(qb, kt):
                            sc = ps_a.tile([P, 512], F32, tag="mm", name="sc")
                            nc.tensor.matmul(
                                sc[:], k_r[:, kt * P:(kt + 1) * P],
                                q_r[h][:, qb * 512:(qb + 1) * 512],
                                start=True, stop=True)
                            pt = ptp.tile([P, 512], F32R, tag="pt")
                            nc.scalar.activation(
                                pt[:], sc[:], mybir.ActivationFunctionType.Exp)
                            if kt >= 4 * qb:
                                nc.vector.tensor_mul(
                                    pt[:], pt[:], mask_sb[:, kt - 4 * qb, :])
                            state[(qb, kt)] = pt

                        def emit_av(qb, kt):
                            pt = state.pop((qb, kt))
                            nkt = 4 * (qb + 1)
                            if kt == 0:
                                state[("av", qb)] = ps_b.tile(
                                    [P, 512], F32, tag="bav", name="av")
                                state[("l", qb)] = ps_c.tile(
                                    [1, 512], F32, tag="lc", name="l")
                            nc.tensor.matmul(
                                state[("av", qb)][:], v_tm[:, kt, :], pt[:],
                                start=(kt == 0), stop=(kt == nkt - 1))
                            nc.tensor.matmul(
                                state[("l", qb)][:], ones_col, pt[:],
                                start=(kt == 0), stop=(kt == nkt - 1))

                        def emit_tail(qb):
                            av_t = state.pop(("av", qb))
                            l_t = state.pop(("l", qb))
                            linv = statp.tile([1, 512], F32R, tag="linv")
                            with nc.allow_low_precision(
                                    reason="f32r is 32-bit storage"):
                                nc.vector.reciprocal(linv[:], l_t[:])
                            bcast = ps_c.tile([P, 512], F32, tag="lc", name="bc")
                            nc.tensor.matmul(
                                bcast[:], ones_row, linv[:],
                                start=True, stop=True)
                            bc_sb = statp.tile([P, 512], F32R, tag="bcs")
                            nc.vector.tensor_copy(bc_sb[:], bcast[:])
                            nc.vector.tensor_mul(
                                attn_f[h][:, qb * 512:(qb + 1) * 512],
                                av_t[:], bc_sb[:])

                        pending_tail = None
                        for i, (qb, kt) in enumerate(sched):
                            emit_scores(qb, kt)
                            if i > 0:
                                pqb, pkt = sched[i - 1]
                                emit_av(pqb, pkt)
                                if pkt == 4 * (pqb + 1) - 1:
                                    pending_tail = pqb
                            if pending_tail is not None and kt == 1:
                                emit_tail(pending_tail)
                                pending_tail = None
                        emit_av(*sched[-1])
                        emit_tail(SC - 1)

                    # ---- o_proj partials ----
                    for tt in range(ST):
                        for dn in range(4):
                            ops = ps_a.tile([P, 512], F32, tag="mm", name="ops")
                            for h in range(QH):
                                nc.tensor.matmul(
                                    ops[:], attn_f[h][:, tt * P:(tt + 1) * P],
                                    wo_sb[:, h, dn * 512:(dn + 1) * 512],
                                    start=(h == 0), stop=(h == QH - 1))
                            ot = oout.tile([P, 512], F32, tag="ot")
                            if (tt * 4 + dn) % 2 == 0:
                                nc.vector.tensor_copy(ot[:], ops[:])
                            else:
                                nc.scalar.copy(ot[:], ops[:])
                            nc.sync.dma_start(
                                PO[toff + tt * P:toff + (tt + 1) * P,
                                   dn * 512:(dn + 1) * 512], ot[:])

            if reps == 1:
                body()
            else:
                with tc.For_i(0, reps, 1):
                    body()
    nc.compile()
    return nc


def attention_host_prep(hidden, cos, sin, ln1_w, wqkv, wo):
    """Builds the 8 per-core input maps for phase 1."""
    x = hidden.reshape(T, D)
    x64 = x.astype(np.float64)
    s = 1.0 / np.sqrt((x64 * x64).mean(-1) + EPS)          # [T] rmsnorm scale
    s32 = s.astype(np.float32)
    xh = np.ascontiguousarray(
        x.T.reshape(DKT, P, T).transpose(1, 0, 2))          # [P, DKT, T]
    wqkv_ln64 = wqkv.astype(np.float64) * ln1_w.astype(np.float64)[None, :]
    wqkv_ln64[: NH * HD] *= 1.0 / np.sqrt(HD)   # fold score scaling into q
    wqkv_ln = wqkv_ln64.astype(np.float32)

    cosT = cos.T.astype(np.float64)                         # [HD, S]
    sinT = sin.T.astype(np.float64)
    pos = np.tile(np.arange(S), B)                          # position of each token
    cs = (cosT[:, pos] * s[None, :]).astype(np.float32)     # [HD, T]
    ss_ = (sinT[:, pos] * s[None, :])
    ss_[:64] *= -1.0                                        # rotate-half sign
    ss_ = ss_.astype(np.float32)
    stok = np.ascontiguousarray(s32.reshape(T // P, P).T)   # [P, T/P]

    # multiplicative causal masks for transposed scores: block row kt within
    # a 512-wide q block, j = kt - 4*qb in 0..3; valid iff r <= c - 128*j
    mk = np.zeros((P, 5, 512), np.float32)
    r = np.arange(P)[:, None]
    c = np.arange(512)[None, :]
    for j in range(4):
        mk[:, j, :] = (r <= c - P * j).astype(np.float32)
    mk[:, 4, :] = 1.0                                       # ones for l / bcast

    in_maps = []
    for cid in range(NCORES):
        rows = np.concatenate([
            np.arange(cid * QH * HD, (cid * QH + QH) * HD),          # q heads
            np.arange(NH * HD + cid * HD, NH * HD + (cid + 1) * HD), # k head
            np.arange((NH + NKV) * HD + cid * HD,
                      (NH + NKV) * HD + (cid + 1) * HD),             # v head
        ])
        wq_c = np.ascontiguousarray(
            wqkv_ln[rows].T.reshape(DKT, P, 4 * P).transpose(1, 0, 2))
        wo_c = np.ascontiguousarray(
            wo[:, cid * QH * HD:(cid + 1) * QH * HD].T
            .reshape(QH, P, D).transpose(1, 0, 2))
        in_maps.append({
            "xh": xh, "wq": wq_c, "wo": wo_c,
            "cs": cs, "ss": ss_, "stok": stok, "masks": mk,
        })
    return in_maps


def run_phase1(hidden, cos, sin, ln1_w, wqkv, wo, reps=1):
    """Returns attn output summed over cores: [T, D] f64."""
    nc = build_phase1(reps)
    in_maps = attention_host_prep(hidden, cos, sin, ln1_w, wqkv, wo)
    res = bass_utils.run_bass_kernel_spmd(nc, in_maps, core_ids=list(range(NCORES)))
    acc = np.zeros((T, D), np.float64)
    for c in range(NCORES):
        acc += res.results[c]["po"].astype(np.float64)
    return acc


# ---------------------------------------------------------------- routing
def route(h2_f64, gate_w):
    """Replicates reference: softmax over experts, top-2, renormalize.
    Returns tok_idx (list of E index arrays) and tok_w (matching weights)."""
    logits = h2_f64 @ gate_w.astype(np.float64).T          # [T, E]
    logits -= logits.max(axis=-1, keepdims=True)
    p = np.exp(logits)
    p /= p.sum(axis=-1, keepdims=True)
    order = np.argsort(-p, axis=-1, kind="stable")[:, :TOPK]   # ties -> lower idx
    tw = np.take_along_axis(p, order, axis=-1)
    tw /= tw.sum(axis=-1, keepdims=True)
    tok_idx, tok_w = [], []
    for e in range(E):
        t_ids, k_ids = np.nonzero(order == e)
        tok_idx.append(t_ids)
        tok_w.append(tw[t_ids, k_ids])
    return tok_idx, tok_w


def moe_host(residual, gate_w, ln2_w, w1, w3, w2, reps=1):
    """Post-attention norm + router + expert dispatch. Returns out [T, D] f32."""
    r64 = residual.astype(np.float64)
    var = (r64 * r64).mean(axis=-1, keepdims=True)
    h2_64 = r64 / np.sqrt(var + EPS) * ln2_w.astype(np.float64)
    h2 = h2_64.astype(np.float32)
    tok_idx, tok_w = route(h2_64, gate_w)
    ys = run_phase2(h2, tok_idx, w1, w3, w2, reps=reps)
    out = np.zeros((T, D), np.float64)
    for e in range(E):
        np.add.at(out, tok_idx[e], tok_w[e][:, None] * ys[e].astype(np.float64))
    return out.astype(np.float32)


# ---------------------------------------------------------------- entry
def kernel(hidden_states, cos, sin, ln1_w, ln2_w, wqkv, wo, gate_w, w1, w3, w2):
    hidden_states = np.asarray(hidden_states, np.float32)
    cos = np.asarray(cos, np.float32)
    sin = np.asarray(sin, np.float32)
    ln1_w = np.asarray(ln1_w, np.float32)
    ln2_w = np.asarray(ln2_w, np.float32)
    wqkv = np.asarray(wqkv, np.float32)
    wo = np.asarray(wo, np.float32)
    gate_w = np.asarray(gate_w, np.float32)
    w1 = np.asarray(w1, np.float32)
    w3 = np.asarray(w3, np.float32)
    w2 = np.asarray(w2, np.float32)

    attn = run_phase1(hidden_states, cos, sin, ln1_w, wqkv, wo)   # [T, D] f64
    residual = (attn + hidden_states.reshape(T, D).astype(np.float64)).astype(np.float32)
    out = moe_host(residual, gate_w, ln2_w, w1, w3, w2)
    return out.reshape(B, S, D), residual.reshape(B, S, D)
